# revision 1
# baseline (speedup 1.0000x reference)
"""Trainium2 Bass kernel for BertSimSelfAttention (sparse_attention).

Problem (full): B=4, M=64, SEQ=256, DIM=1024, H=16, HD=64.
Effective batch rows R = B*SEQ = 1024, each row: m=64 tokens of dim=1024.
  hs  = transpose(hidden_states,(0,2,1,3)).reshape(R, 64, 1024)
  q/k/v = hs @ W{q,k,v}.T + b   (per token)
  per (row, head): scores = (q @ k.T)/8 * sim[row] + (-1e4)*(1-am[row,j])
  probs = softmax_j(scores);  ctx = probs @ v  -> out [R, 64, 1024]

Sharding: data-parallel over rows, 128 rows/core x 8 cores. The host
pre-transposes x, W (and sim per row) so the device consumes
contraction-major layouts directly; the 1/sqrt(hd) scale is folded into
Wq/bq on the host.

Per-core design (measured ~1.0 ms on 8 axon-tunneled TRN2 cores):
  - xT [d, t] and WT [d, o] loaded d-major, rounded to fp32r on DVE.
  - Projections in fp32r (1 cyc/row on PE, ~tf32 accuracy): qT/kT in
    [o, t] layout bf16 (heads on partition strips by head parity),
    v natural [t, o] bf16. q/k bias added by ACT Identity at
    evacuation; v bias accumulated as a K=1 fp32r matmul (skipped
    entirely when bv == 0).
  - scores TRANSPOSED: S'[j, q] per (row, head) via bf16 paired
    matmuls (tile_position strips by head parity) into one PSUM bank
    [128 = 2x64 j, 512 = 8 head-pairs x 64 q] (fp32).
  - softmax over j (= partitions): t = S' * simT (DVE, sim repeated
    via stride-0 AP); e = exp(t + maskcol) on ACT, where the additive
    key mask is a per-partition bias column (masked lanes give exactly
    0); denominators via ONE PE matmul with a block-ones stationary
    (sums each 64-half and broadcasts to its partitions); in-place
    reciprocal_approx_fast on PSUM; probs.T = e * recip (bf16).
  - probs.T slices feed ctx matmuls directly as stationary (no
    transposes); v duplicated to both partition strips by SBUF DMA;
    ctx pairs (head-even, head-odd) per row into one PSUM bank.
  - Emission software-pipelines: tile i's projection groups are
    interleaved with tile (i-1)'s attention rows (split into
    scores/softmax and ctx units with one-row lookahead) so the PE
    stream stays dense and HAM stays warm.
"""

import sys

sys.path.insert(0, "/opt/trn_rl_repo")

import numpy as np
import concourse.bass as bass
import concourse.bacc as bacc
import concourse.mybir as mybir
import concourse.tile as tile

F32 = mybir.dt.float32
F32R = mybir.dt.float32r
BF16 = mybir.dt.bfloat16
AF = mybir.ActivationFunctionType
ALU = mybir.AluOpType

N_CORES = 8
M = 64                    # tokens per row
DIM = 1024
H = 16
HD = 64
NEG = -10000.0


def build_core_kernel(nc, n_tiles=16, rows_per_tile=8, debug=False, use_bv=True):
    """Emit the per-core program. tile = rows_per_tile rows (must be even)."""
    T_TILE = rows_per_tile * M        # tokens per tile (512 default)
    n_rows = n_tiles * rows_per_tile
    n_tok = n_rows * M
    SUB = T_TILE // 128               # 128-token subtiles per tile

    xt_d = nc.dram_tensor("xT", (DIM, n_tok), F32, kind="ExternalInput")
    sim_d = nc.dram_tensor("simg", (n_rows, M, M), F32, kind="ExternalInput")
    am_d = nc.dram_tensor("am", (n_rows, M), F32, kind="ExternalInput")
    wq_d = nc.dram_tensor("WqT", (DIM, DIM), F32, kind="ExternalInput")
    wk_d = nc.dram_tensor("WkT", (DIM, DIM), F32, kind="ExternalInput")
    wv_d = nc.dram_tensor("WvT", (DIM, DIM), F32, kind="ExternalInput")
    bq_d = nc.dram_tensor("bq", (DIM,), F32, kind="ExternalInput")
    bk_d = nc.dram_tensor("bk", (DIM,), F32, kind="ExternalInput")
    bv_d = nc.dram_tensor("bv", (DIM,), F32, kind="ExternalInput")
    id_d = nc.dram_tensor("ident", (128, 128), F32, kind="ExternalInput")
    sel_d = nc.dram_tensor("selm", (128, 2), F32, kind="ExternalInput")
    bsel_d = nc.dram_tensor("bselm", (2, 128), F32, kind="ExternalInput")
    out_d = nc.dram_tensor("out", (n_tok, DIM), F32, kind="ExternalOutput")

    dbg = {}
    if debug:
        dbg["qt"] = nc.dram_tensor("dbg_qt", (DIM, n_tok), F32, kind="ExternalOutput")
        dbg["kt"] = nc.dram_tensor("dbg_kt", (DIM, n_tok), F32, kind="ExternalOutput")
        dbg["v"] = nc.dram_tensor("dbg_v", (n_tok, DIM), F32, kind="ExternalOutput")
        dbg["pr"] = nc.dram_tensor("dbg_pr", (n_rows, 128, 512), F32,
                                   kind="ExternalOutput")
        dbg["s"] = nc.dram_tensor("dbg_s", (n_rows, 128, 512), F32,
                                  kind="ExternalOutput")

    with tile.TileContext(nc) as tc:
        with (
            tc.tile_pool(name="consts", bufs=1) as consts,
            tc.tile_pool(name="stage", bufs=3) as stage,
            tc.tile_pool(name="xtp", bufs=2) as xtp,
            tc.tile_pool(name="qkp", bufs=2) as qkp,
            tc.tile_pool(name="vp", bufs=2) as vp,
            tc.tile_pool(name="rowp", bufs=2) as rowp,
            tc.tile_pool(name="small_ps", bufs=2, space="PSUM") as small_ps,
            tc.tile_pool(name="proj_ps", bufs=2, space="PSUM") as proj_ps,
            tc.tile_pool(name="att_ps", bufs=4, space="PSUM") as att_ps,
        ):
            # ---------------- tiny consts first ----------------
            ident = consts.tile([128, 128], F32)
            nc.sync.dma_start(ident[:], id_d[:])

            am_all = consts.tile([128, M], F32)
            if n_rows < 128:
                nc.gpsimd.memset(am_all[:], 1.0)
            nc.sync.dma_start(am_all[0:n_rows, :], am_d[:])

            # block-ones selector: halfones[p, m] = 1 iff same 64-half;
            # halfones.T @ e sums each half and broadcasts to its partitions
            halfones = consts.tile([128, 128], BF16)
            nc.gpsimd.memset(halfones[:], 0.0)
            nc.gpsimd.memset(halfones[0:64, 0:64], 1.0)
            nc.gpsimd.memset(halfones[64:128, 64:128], 1.0)

            bq_sb = consts.tile([128, 8], F32)
            bk_sb = consts.tile([128, 8], F32)
            nc.sync.dma_start(bq_sb[:], bq_d[:].rearrange("(o p) -> p o", p=128))
            nc.sync.dma_start(bk_sb[:], bk_d[:].rearrange("(o p) -> p o", p=128))

            if use_bv:
                # bv as a K=1 fp32r pair for psum-accumulate
                ones_f = consts.tile([1, 128], F32)
                nc.gpsimd.memset(ones_f[:], 1.0)
                ones_r = consts.tile([1, 128], F32R)
                nc.vector.tensor_copy(ones_r[:], ones_f[:])
                bv_row = consts.tile([1, DIM], F32)
                nc.sync.dma_start(bv_row[:],
                                  bv_d[:].rearrange("(a o) -> a o", a=1))
                bv_r = consts.tile([1, DIM], F32R)
                nc.vector.tensor_copy(bv_r[:], bv_row[:])

            # mask bias columns: mcolT2[:, r] = -1e4*(1 - am[r, j]) on both
            # partition halves (exp-bias per key token j)
            mcolT2 = consts.tile([128, 128], F32)
            amt_ps = small_ps.tile([128, 128], F32, tag="srb")
            nc.tensor.transpose(amt_ps[0:M, 0:128], am_all[:], ident[:])
            nc.vector.tensor_scalar(
                mcolT2[0:64, :], amt_ps[0:M, 0:128], -NEG, NEG,
                op0=ALU.mult, op1=ALU.add)
            nc.vector.tensor_scalar(
                mcolT2[64:128, :], amt_ps[0:M, 0:128], -NEG, NEG,
                op0=ALU.mult, op1=ALU.add)

            # ---------------- weights (+ tile-0 x interleaved) ----------
            def emit_xt(ti):
                t0 = ti * T_TILE
                xt = [xtp.tile([128, T_TILE], F32R, tag=f"xt{d}",
                               name=f"xt{d}_{ti}") for d in range(8)]
                for dch in range(8):
                    xst = stage.tile([128, T_TILE], F32, tag="xstage",
                                     name=f"xst{dch}_{ti}")
                    nc.sync.dma_start(
                        xst[:], xt_d[128 * dch:128 * dch + 128, t0:t0 + T_TILE]
                    )
                    nc.vector.tensor_copy(xt[dch][:], xst[:])
                return xt

            def emit_w(name, w_d, dchs):
                wt = wts[name]
                for dch in dchs:
                    for hh in range(DIM // 512):
                        wnat = stage.tile([128, 512], F32, tag="xstage",
                                          name=f"wn{name}{dch}{hh}")
                        weng = nc.gpsimd if hh == 0 else nc.sync
                        weng.dma_start(
                            wnat[:],
                            w_d[128 * dch:128 * dch + 128,
                                512 * hh:512 * hh + 512])
                        nc.vector.tensor_copy(
                            wt[dch][:, 512 * hh:512 * hh + 512], wnat[:])

            wts = {name: [consts.tile([128, DIM], F32R, tag=f"w{name}{d}",
                                      name=f"w{name}{d}") for d in range(8)]
                   for name in ("q", "k", "v")}
            xt0 = [xtp.tile([128, T_TILE], F32R, tag=f"xt{d}",
                             name=f"xt{d}_0") for d in range(8)]
            for dch in range(8):
                emit_w("q", wq_d, [dch])
                xst = stage.tile([128, T_TILE], F32, tag="xstage",
                                 name=f"xst{dch}_0")
                nc.sync.dma_start(xst[:], xt_d[128 * dch:128 * dch + 128,
                                               0:T_TILE])
                nc.vector.tensor_copy(xt0[dch][:], xst[:])
            emit_w("k", wk_d, range(8))
            emit_w("v", wv_d, range(8))
            wqt, wkt, wvt = wts["q"], wts["k"], wts["v"]

            # ---------------- main loop over token tiles ----------------
            # Emission interleaves tile ti's projection groups with tile
            # (ti-1)'s attention rows so the PE program order has dense
            # matmul work to fill softmax dependency stalls (keeps HAM warm).

            def make_proj(ti, xt):
                qt = [qkp.tile([128, T_TILE], BF16, tag=f"qt{o}",
                               name=f"qt{o}_{ti}") for o in range(8)]
                kt = [qkp.tile([128, T_TILE], BF16, tag=f"kt{o}",
                               name=f"kt{o}_{ti}") for o in range(8)]
                vts = [vp.tile([128, DIM], BF16, tag=f"v{s}",
                               name=f"v{s}_{ti}") for s in range(SUB)]
                groups = []

                def qk_group(wt, dst, b_sb, och):
                    ps = proj_ps.tile([128, T_TILE], F32, tag="proj",
                                      name=f"qkps{och}_{ti}")
                    for dch in range(8):
                        nc.tensor.matmul(
                            ps[:],
                            wt[dch][:, 128 * och:128 * och + 128],
                            xt[dch][:],
                            start=(dch == 0), stop=(dch == 7),
                        )
                    nc.scalar.activation(
                        dst[och][:], ps[:], AF.Identity,
                        bias=b_sb[:, och:och + 1], scale=1.0,
                    )

                def v_group(sub, oh):
                    vt = vts[sub]
                    ps = proj_ps.tile([128, 512], F32, tag="proj",
                                      name=f"vps{sub}{oh}_{ti}")
                    sl = slice(512 * oh, 512 * oh + 512)
                    for dch in range(8):
                        nc.tensor.matmul(
                            ps[:],
                            xt[dch][:, 128 * sub:128 * sub + 128],
                            wvt[dch][:, 512 * oh:512 * oh + 512],
                            start=(dch == 0), stop=(dch == 7) and not use_bv,
                        )
                    if use_bv:
                        nc.tensor.matmul(
                            ps[:], ones_r[:], bv_r[:, sl],
                            start=False, stop=True,
                        )
                    nc.scalar.copy(vt[:, sl], ps[:])

                for wt, dst, b_sb in ((wqt, qt, bq_sb), (wkt, kt, bk_sb)):
                    for och in range(8):
                        groups.append(
                            lambda wt=wt, dst=dst, b_sb=b_sb, och=och:
                            qk_group(wt, dst, b_sb, och))
                for sub in range(SUB):
                    for oh in range(2):
                        groups.append(lambda sub=sub, oh=oh: v_group(sub, oh))
                return qt, kt, vts, groups

            def make_att_rows(ti, qt, kt, vts, la=1):
                rowstate = {}

                def att_row_a(rr):
                    r = ti * rows_per_tile + rr

                    simt2 = rowp.tile([128, M], F32, tag="sim2",
                                      name=f"sim2_{r}")
                    nc.gpsimd.dma_start(simt2[0:64, :], sim_d[r, :, :])
                    nc.gpsimd.dma_start(simt2[64:128, :], sim_d[r, :, :])

                    # scores transposed: S'[j, q] (bf16 in, fp32 psum)
                    s_ps = att_ps.tile([128, 512], F32, tag="att",
                                       name=f"s_{r}")
                    tsl = slice(M * rr, M * rr + M)
                    for h in range(H):
                        hp, half = h // 2, h % 2
                        st = 64 * half
                        nc.tensor.matmul(
                            s_ps[st:st + 64, 64 * hp:64 * hp + 64],
                            kt[h // 2][st:st + 64, tsl],
                            qt[h // 2][st:st + 64, tsl],
                            start=True, stop=True,
                            tile_position=(st, st),
                        )

                    # t = S' * simT;  e = exp(t + maskcol)  (bf16 out)
                    tt = rowp.tile([128, 512], F32, tag="tt", name=f"tt_{r}")
                    nc.vector.tensor_tensor(
                        tt[:].rearrange("p (a j) -> p a j", j=M),
                        s_ps[:].rearrange("p (a j) -> p a j", j=M),
                        simt2[:].rearrange("p (a j) -> p a j", a=1)
                        .broadcast_to([128, 8, M]),
                        op=ALU.mult,
                    )
                    et = rowp.tile([128, 512], BF16, tag="et", name=f"et_{r}")
                    nc.scalar.activation(et[:], tt[:], AF.Exp,
                                         bias=mcolT2[:, r:r + 1])

                    # denominators summed + broadcast in one PE matmul
                    dn_ps = small_ps.tile([128, 512], F32, tag="srb",
                                          name=f"dn_{r}")
                    nc.tensor.matmul(dn_ps[:], halfones[:], et[:],
                                     start=True, stop=True)

                    # in-place approx reciprocal on PSUM (~18 bits), then
                    # probs.T = e * (1/denom) (bf16) -- direct ctx stationary
                    nc.vector.reciprocal_approx_fast(out=dn_ps[:], in_=dn_ps[:])
                    pt = rowp.tile([128, 512], BF16, tag="pt", name=f"pt_{r}")
                    nc.vector.tensor_tensor(pt[:], et[:], dn_ps[:],
                                            op=ALU.mult)
                    if debug:
                        dpr = stage.tile([128, 512], F32, tag="dbgpr",
                                         name=f"dpr_{r}")
                        nc.scalar.copy(dpr[:], pt[:])
                        nc.gpsimd.dma_start(dbg["pr"][r, :, :], dpr[:])
                        ssb = stage.tile([128, 512], F32, tag="ssb",
                                         name=f"ssb_{r}")
                        nc.scalar.copy(ssb[:], s_ps[:])
                        nc.gpsimd.dma_start(dbg["s"][r, :, :], ssb[:])
                    rowstate[rr] = pt

                def att_row_b(rr):
                    r = ti * rows_per_tile + rr
                    rp = rr % 2
                    pt = rowstate.pop(rr)

                    # v duplicated to both partition strips (SBUF->SBUF DMA)
                    vt = vts[rr // 2]
                    v2 = rowp.tile([128, DIM], BF16, tag="v2", name=f"v2_{r}")
                    nc.gpsimd.dma_start(v2[0:64, :],
                                        vt[64 * rp:64 * rp + 64, :])
                    nc.gpsimd.dma_start(v2[64:128, :],
                                        vt[64 * rp:64 * rp + 64, :])

                    # ctx: head pairs (even, odd) at partition strips (0, 64)
                    ctx_ps = att_ps.tile([128, 512], F32, tag="att",
                                         name=f"ctx_{r}")
                    for hp in range(8):
                        for par in range(2):
                            h = 2 * hp + par
                            st = 64 * par
                            nc.tensor.matmul(
                                ctx_ps[st:st + 64, 64 * hp:64 * hp + 64],
                                pt[st:st + 64, 64 * hp:64 * hp + 64],
                                v2[st:st + 64, 64 * h:64 * h + 64],
                                start=True, stop=True,
                                tile_position=(st, st),
                            )
                    osb = rowp.tile([128, 512], F32, tag="osb",
                                    name=f"osb_{r}", bufs=1)
                    nc.scalar.copy(osb[:], ctx_ps[:])
                    # out[64r + q, 64h + hd]; strip par holds heads 2hp+par
                    ov = out_d[M * r:M * r + M, :].rearrange(
                        "q (hp two hd) -> q hp two hd", two=2, hd=64)
                    for par in range(2):
                        nc.sync.dma_start(
                            ov[:, :, par, :],
                            osb[64 * par:64 * par + 64, :]
                            .rearrange("q (hp hd) -> q hp hd", hd=64),
                        )

                # lookahead pipeline (la rows of softmax chain in flight
                # over each row's ctx matmuls)
                units = [lambda rr=rr: att_row_a(rr) for rr in range(la)]
                for rr in range(la, rows_per_tile):
                    units.append(lambda rr=rr: att_row_a(rr))
                    units.append(lambda rr=rr: att_row_b(rr - la))
                for rr in range(rows_per_tile - la, rows_per_tile):
                    units.append(lambda rr=rr: att_row_b(rr))
                return units

            prev_rows = []
            for ti in range(n_tiles):
                xt = xt0 if ti == 0 else emit_xt(ti)
                qt, kt, vts, groups = make_proj(ti, xt)
                ri = 0
                for gi, g in enumerate(groups):
                    g()
                    while (ri < len(prev_rows)
                           and (gi + 1) * len(prev_rows) // len(groups) > ri):
                        prev_rows[ri]()
                        ri += 1
                while ri < len(prev_rows):
                    prev_rows[ri]()
                    ri += 1
                prev_rows = make_att_rows(
                    ti, qt, kt, vts, la=(2 if ti == n_tiles - 1 else 1))
            for row in prev_rows:
                row()

    return dict(out=out_d)


def _prepare_shards(hidden_states, attention_mask, sim_graph, Wq, bq, Wk, bk, Wv, bv,
                    n_cores=N_CORES):
    b, m, seq, dim = hidden_states.shape
    R = b * seq
    hs = np.transpose(np.asarray(hidden_states), (0, 2, 1, 3)).reshape(R, m, dim)
    am = np.ascontiguousarray(
        np.transpose(np.asarray(attention_mask), (0, 2, 1)).reshape(R, m),
        dtype=np.float32)
    sim = np.ascontiguousarray(
        np.transpose(np.asarray(sim_graph), (0, 2, 1)), dtype=np.float32)
    ident = np.eye(128, dtype=np.float32)
    selm = np.zeros((128, 2), np.float32)
    selm[0:64, 0] = 1.0
    selm[64:128, 1] = 1.0
    bselm = np.zeros((2, 128), np.float32)
    bselm[0, 0:64] = 1.0
    bselm[1, 64:128] = 1.0
    WqT = np.ascontiguousarray(np.asarray(Wq).T * 0.125, np.float32)
    WkT = np.ascontiguousarray(np.asarray(Wk).T, np.float32)
    WvT = np.ascontiguousarray(np.asarray(Wv).T, np.float32)
    rows_per_core = R // n_cores
    in_maps = []
    for c in range(n_cores):
        r0 = c * rows_per_core
        xT = np.ascontiguousarray(
            hs[r0:r0 + rows_per_core].reshape(rows_per_core * m, dim).T,
            np.float32)
        in_maps.append(dict(
            xT=xT,
            simg=sim[r0:r0 + rows_per_core],
            am=am[r0:r0 + rows_per_core],
            WqT=WqT, WkT=WkT, WvT=WvT,
            bq=np.ascontiguousarray(np.asarray(bq) * 0.125, np.float32),
            bk=np.ascontiguousarray(bk, np.float32),
            bv=np.ascontiguousarray(bv, np.float32),
            ident=ident, selm=selm, bselm=bselm,
        ))
    return in_maps


_CACHE = {}


def _get_compiled(use_bv=True):
    key = ("nc", use_bv)
    if key not in _CACHE:
        nc = bacc.Bacc("TRN2", target_bir_lowering=False, debug=False)
        build_core_kernel(nc, use_bv=use_bv)
        nc.compile()
        _CACHE[key] = nc
    return _CACHE[key]


LAST_EXEC_NS = [None]


def kernel(hidden_states, attention_mask, sim_graph, Wq, bq, Wk, bk, Wv, bv,
           b=4, m=64, seq=256, dim=1024, **_):
    import os
    from concourse.bass_utils import run_bass_kernel_spmd

    use_bv = bool(np.any(np.asarray(bv)))
    nc = _get_compiled(use_bv=use_bv)
    in_maps = _prepare_shards(hidden_states, attention_mask, sim_graph,
                              Wq, bq, Wk, bk, Wv, bv)
    trace = bool(int(os.environ.get("BERT_TRACE", "0")))
    if trace:
        try:  # register the NTFF hook if the middleware didn't
            from antenv.axon_hooks import (get_axon_ntff_profile_hook,
                                           set_axon_ntff_profile_hook)
            if get_axon_ntff_profile_hook() is None:
                from trn_agent_boot.trn_boot import _ntff_profile_via_ctypes
                set_axon_ntff_profile_hook(
                    _ntff_profile_via_ctypes("/opt/axon/libaxon_pjrt.so"))
        except Exception:
            trace = False
    res = run_bass_kernel_spmd(nc, in_maps, list(range(N_CORES)), trace=trace)
    LAST_EXEC_NS[0] = res.exec_time_ns
    R = int(b) * int(seq)
    out = np.concatenate([res.results[c]["out"] for c in range(N_CORES)], axis=0)
    return out.reshape(R, int(m), int(dim))



# revision 2
# speedup vs baseline: 1.0515x; 1.0515x over previous
"""Trainium2 Bass kernel for BertSimSelfAttention — v2 (pair-row layout).

Problem (full): B=4, M=64, SEQ=256, DIM=1024, H=16, HD=64.
R = B*SEQ = 1024 rows, data-parallel 128 rows/core x 8 cores.

v2 design vs v1:
  - x and W shipped bf16 (halved DMA, no on-device f32r converts, FWL
    eligible stationaries for the projections).
  - Attention processes ROW PAIRS with row-parity partition strips:
    S'/e/pt for row r live at partitions 64*(r%2)..+64, all 16 heads on
    the free axis (2 PSUM banks per pair). v is consumed at its natural
    partition strip — the per-row v duplication DMA (33.5 MB) is gone.
  - Scores: one matmul per (row, head-pair): stationary = kt[hp] full
    [128,64] (both heads' dims), moving = zero-padded qbd [128,(2,64)]
    so each head contracts only its own 64 dims. Halves scores LDWEIGHTS
    columns and instruction count.
  - Output: heads land in natural order; one contiguous [128, 4KB] DMA
    per pair.
  - Softmax: t = S'*simT (DVE), e = exp(t + mcol) (ACT, mask bias per
    partition, host-built mcol), denominators via halfones matmul,
    reciprocal_approx_fast in PSUM, pt = e*recip (DVE, bf16).
"""

import sys

sys.path.insert(0, "/opt/trn_rl_repo")

import numpy as np
import concourse.bass as bass
import concourse.bacc as bacc
import concourse.mybir as mybir
import concourse.tile as tile

F32 = mybir.dt.float32
BF16 = mybir.dt.bfloat16
AF = mybir.ActivationFunctionType
ALU = mybir.AluOpType

N_CORES = 8
M = 64
DIM = 1024
H = 16
HD = 64
NEG = -10000.0


def build_core_kernel(nc, n_tiles=16, rows_per_tile=8, debug=False,
                      use_bq=False, use_bk=False, use_bv=False):
    assert rows_per_tile % 2 == 0
    T = rows_per_tile * M              # tokens per tile (512)
    n_rows = n_tiles * rows_per_tile
    n_tok = n_rows * M
    n_pairs = n_rows // 2
    PPT = rows_per_tile // 2           # pairs per tile
    SUB = T // 128                     # 128-token subtiles per tile

    xt_d = nc.dram_tensor("xT", (DIM, n_tok), BF16, kind="ExternalInput")
    sim_d = nc.dram_tensor("simT", (n_rows, M, M), F32, kind="ExternalInput")
    mcol_d = nc.dram_tensor("mcol", (128, n_pairs), F32, kind="ExternalInput")
    wq_d = nc.dram_tensor("WqT", (DIM, DIM), BF16, kind="ExternalInput")
    wk_d = nc.dram_tensor("WkT", (DIM, DIM), BF16, kind="ExternalInput")
    wv_d = nc.dram_tensor("WvT", (DIM, DIM), BF16, kind="ExternalInput")
    bq_d = nc.dram_tensor("bq", (DIM,), F32, kind="ExternalInput")
    bk_d = nc.dram_tensor("bk", (DIM,), F32, kind="ExternalInput")
    bv_d = nc.dram_tensor("bv", (DIM,), F32, kind="ExternalInput")
    hones_d = nc.dram_tensor("hones", (128, 128), BF16, kind="ExternalInput")
    out_d = nc.dram_tensor("out", (n_tok, DIM), F32, kind="ExternalOutput")

    dbg = {}
    if debug:
        dbg["qbd"] = nc.dram_tensor("dbg_qbd", (8, 128, 2 * T), F32,
                                    kind="ExternalOutput")
        dbg["kt"] = nc.dram_tensor("dbg_kt", (8, 128, T), F32,
                                   kind="ExternalOutput")
        dbg["v"] = nc.dram_tensor("dbg_v", (SUB, 128, DIM), F32,
                                  kind="ExternalOutput")
        dbg["et"] = nc.dram_tensor("dbg_et", (n_pairs, 128, 1024), F32,
                                   kind="ExternalOutput")
        dbg["pt"] = nc.dram_tensor("dbg_pt", (n_pairs, 128, 1024), F32,
                                   kind="ExternalOutput")

    with tile.TileContext(nc) as tc:
        with (
            tc.tile_pool(name="consts", bufs=1) as consts,
            tc.tile_pool(name="xtp", bufs=2) as xtp,
            tc.tile_pool(name="qbdp", bufs=1) as qbdp,
            tc.tile_pool(name="ktp", bufs=2) as ktp,
            tc.tile_pool(name="vp", bufs=2) as vp,
            tc.tile_pool(name="rowp", bufs=2) as rowp,
            tc.tile_pool(name="proj_ps", bufs=3, space="PSUM") as proj_ps,
            tc.tile_pool(name="att_ps", bufs=3, space="PSUM") as att_ps,
            tc.tile_pool(name="ctx_ps", bufs=2, space="PSUM") as ctx_ps,
        ):
            # ---------------- consts ----------------
            hones = consts.tile([128, 128], BF16)
            nc.sync.dma_start(hones[:], hones_d[:])

            mcol = consts.tile([128, n_pairs], F32)
            nc.sync.dma_start(mcol[:], mcol_d[:])

            if use_bq or use_bk:
                bq_sb = consts.tile([128, 8], F32)
                bk_sb = consts.tile([128, 8], F32)
                nc.sync.dma_start(bq_sb[:], bq_d[:].rearrange("(o p) -> p o", p=128))
                nc.sync.dma_start(bk_sb[:], bk_d[:].rearrange("(o p) -> p o", p=128))
            if use_bv:
                ones_f = consts.tile([1, 128], F32)
                nc.gpsimd.memset(ones_f[:], 1.0)
                ones_b = consts.tile([1, 128], BF16)
                nc.vector.tensor_copy(ones_b[:], ones_f[:])
                bv_row = consts.tile([1, DIM], F32)
                nc.sync.dma_start(bv_row[:], bv_d[:].rearrange("(a o) -> a o", a=1))
                bv_b = consts.tile([1, DIM], BF16)
                nc.vector.tensor_copy(bv_b[:], bv_row[:])

            # qbd: persistent ping/pong zero-padded tiles. The off-diagonal
            # quadrants are zeroed once here; projections only ever write the
            # two valid quadrants, so the zeros persist across tiles.
            qbd_sets = [[qbdp.tile([128, 2 * T], BF16, tag=f"q{hp}{s}",
                                   name=f"qbd{hp}_{s}") for hp in range(8)]
                        for s in range(2)]
            for s in range(2):
                for hp in range(8):
                    nc.gpsimd.memset(qbd_sets[s][hp][:], 0.0)

            # v_bd: per-subtile zero-padded v for merged-row ctx matmuls.
            # Layout [128 (tok: row0 strip0 / row1 strip1), 16 h, 2 two, 64]:
            # quadrant (strip0, two=1) and (strip1, two=0) stay zero.
            vbd_sets = [[qbdp.tile([128, H * 128], BF16, tag=f"vb{sub}{s}",
                                   name=f"vbd{sub}_{s}") for sub in range(SUB)]
                        for s in range(2)]
            for s in range(2):
                for sub in range(SUB):
                    nc.gpsimd.memset(vbd_sets[s][sub][:], 0.0)

            # ---------------- emit helpers ----------------
            def emit_xt(ti):
                t0 = ti * T
                xt = [xtp.tile([128, T], BF16, tag=f"xt{d}", name=f"xt{d}_{ti}")
                      for d in range(8)]
                for dch in range(8):
                    nc.sync.dma_start(
                        xt[dch][:], xt_d[128 * dch:128 * dch + 128, t0:t0 + T])
                return xt

            wts = {name: [consts.tile([128, DIM], BF16, tag=f"w{name}{d}",
                                      name=f"w{name}{d}") for d in range(8)]
                   for name in ("q", "k", "v")}

            def emit_w(name, w_d, dchs, eng=None):
                wt = wts[name]
                for dch in dchs:
                    (eng or nc.sync).dma_start(
                        wt[dch][:], w_d[128 * dch:128 * dch + 128, :])

            def make_proj(ti, xt):
                qbd = qbd_sets[ti % 2]
                kt = [ktp.tile([128, T], BF16, tag=f"k{hp}",
                               name=f"kt{hp}_{ti}") for hp in range(8)]
                vts = vbd_sets[ti % 2]
                groups = []

                def q_group(och):
                    ps = proj_ps.tile([128, T], F32, tag="proj",
                                      name=f"qps{och}_{ti}")
                    for dch in range(8):
                        nc.tensor.matmul(
                            ps[:], wts["q"][dch][:, 128 * och:128 * och + 128],
                            xt[dch][:], start=(dch == 0), stop=(dch == 7))
                    dst = qbd[och]
                    if use_bq:
                        nc.scalar.activation(dst[0:64, 0:T], ps[0:64, :],
                                             AF.Identity,
                                             bias=bq_sb[0:64, och:och + 1],
                                             scale=1.0)
                        nc.vector.tensor_tensor(
                            dst[64:128, T:2 * T], ps[64:128, :],
                            bq_sb[64:128, och:och + 1]
                            .rearrange("p a -> p a").broadcast_to([64, T]),
                            op=ALU.add)
                    else:
                        nc.scalar.copy(dst[0:64, 0:T], ps[0:64, :])
                        nc.vector.tensor_copy(dst[64:128, T:2 * T], ps[64:128, :])

                def k_group(och):
                    ps = proj_ps.tile([128, T], F32, tag="proj",
                                      name=f"kps{och}_{ti}")
                    for dch in range(8):
                        nc.tensor.matmul(
                            ps[:], wts["k"][dch][:, 128 * och:128 * och + 128],
                            xt[dch][:], start=(dch == 0), stop=(dch == 7))
                    if use_bk:
                        nc.scalar.activation(kt[och][:], ps[:], AF.Identity,
                                             bias=bk_sb[:, och:och + 1],
                                             scale=1.0)
                    else:
                        nc.scalar.copy(kt[och][:], ps[:])

                def v_group(sub, oh):
                    ps = proj_ps.tile([128, 512], F32, tag="proj",
                                      name=f"vps{sub}{oh}_{ti}")
                    sl = slice(512 * oh, 512 * oh + 512)
                    for dch in range(8):
                        nc.tensor.matmul(
                            ps[:], xt[dch][:, 128 * sub:128 * sub + 128],
                            wts["v"][dch][:, sl],
                            start=(dch == 0), stop=(dch == 7) and not use_bv)
                    if use_bv:
                        nc.tensor.matmul(ps[:], ones_b[:], bv_b[:, sl],
                                         start=False, stop=True)
                    vb = vts[sub][:].rearrange("p (h two hd) -> p h two hd",
                                               two=2, hd=64)
                    hsl = slice(8 * oh, 8 * oh + 8)
                    nc.scalar.copy(
                        vb[0:64, hsl, 0, :],
                        ps[0:64, :].rearrange("p (h hd) -> p h hd", hd=64))
                    nc.vector.tensor_copy(
                        vb[64:128, hsl, 1, :],
                        ps[64:128, :].rearrange("p (h hd) -> p h hd", hd=64))

                for och in range(8):
                    groups.append(lambda och=och: q_group(och))
                for och in range(8):
                    groups.append(lambda och=och: k_group(och))
                for sub in range(SUB):
                    for oh in range(2):
                        groups.append(lambda sub=sub, oh=oh: v_group(sub, oh))
                return qbd, kt, vts, groups

            # ---------------- attention (pair units) ----------------
            def make_att(ti, qbd, kt, vts, la=1):
                state = {}

                def att_a(pr):
                    P = ti * PPT + pr          # global pair index

                    simP = rowp.tile([128, M], F32, tag="sim", name=f"sim_{P}")
                    nc.sync.dma_start(simP[0:64, :], sim_d[2 * P, :, :])
                    nc.sync.dma_start(simP[64:128, :], sim_d[2 * P + 1, :, :])

                    sA = att_ps.tile([128, 512], F32, tag="att", name=f"sA_{P}")
                    sB = att_ps.tile([128, 512], F32, tag="att", name=f"sB_{P}")
                    banks = (sA, sB)
                    for rp in range(2):
                        rr = 2 * pr + rp
                        tsl = slice(M * rr, M * rr + M)
                        q2 = slice(2 * M * rr, 2 * M * rr + M)  # unused; doc
                        for hp in range(8):
                            bank = banks[hp // 4]
                            c0 = 128 * (hp % 4)
                            nc.tensor.matmul(
                                bank[64 * rp:64 * rp + 64, c0:c0 + 128],
                                kt[hp][:, tsl],
                                qbd[hp][:]
                                .rearrange("p (two t) -> p two t", two=2)
                                [:, :, tsl],
                                start=True, stop=True)

                    # t = S' * simT ; e = exp(t + mcol)
                    tt = rowp.tile([128, 1024], F32, tag="tt", name=f"tt_{P}")
                    et = rowp.tile([128, 1024], BF16, tag="et", name=f"et_{P}")
                    for b in range(2):
                        nc.vector.tensor_tensor(
                            tt[:, 512 * b:512 * b + 512]
                            .rearrange("p (a j) -> p a j", j=M),
                            banks[b][:].rearrange("p (a j) -> p a j", j=M),
                            simP[:].rearrange("p (a j) -> p a j", a=1)
                            .broadcast_to([128, 8, M]),
                            op=ALU.mult)
                    nc.scalar.activation(et[:], tt[:], AF.Exp,
                                         bias=mcol[:, P:P + 1])

                    # NOTE: simP is indexed [j, q] per strip; matches S'[j, q].
                    dnA = att_ps.tile([128, 512], F32, tag="att", name=f"dnA_{P}")
                    dnB = att_ps.tile([128, 512], F32, tag="att", name=f"dnB_{P}")
                    for b, dn in enumerate((dnA, dnB)):
                        nc.tensor.matmul(dn[:], hones[:],
                                         et[:, 512 * b:512 * b + 512],
                                         start=True, stop=True)
                        nc.vector.reciprocal_approx_fast(out=dn[:], in_=dn[:])

                    pt = rowp.tile([128, 1024], BF16, tag="pt", name=f"pt_{P}")
                    for b, dn in enumerate((dnA, dnB)):
                        nc.vector.tensor_tensor(pt[:, 512 * b:512 * b + 512],
                                                et[:, 512 * b:512 * b + 512],
                                                dn[:], op=ALU.mult)
                    if debug:
                        dsb = rowp.tile([128, 1024], F32, tag="dbg",
                                        name=f"de_{P}")
                        nc.scalar.copy(dsb[:], et[:])
                        nc.gpsimd.dma_start(dbg["et"][P, :, :], dsb[:])
                        dpb = rowp.tile([128, 1024], F32, tag="dbg2",
                                        name=f"dp_{P}")
                        nc.scalar.copy(dpb[:], pt[:])
                        nc.gpsimd.dma_start(dbg["pt"][P, :, :], dpb[:])
                    state[pr] = pt

                def att_b(pr):
                    P = ti * PPT + pr
                    pt = state.pop(pr)
                    vb = vts[pr][:].rearrange("p (h two hd) -> p h two hd",
                                              two=2, hd=64)
                    # in the last tile the proj psum pool is idle — borrow it
                    # so consecutive pairs' ctx don't serialize on 2 banks
                    pool = (proj_ps if (ti == n_tiles - 1 and pr % 2 == 1)
                            else ctx_ps)
                    ctag = "proj" if pool is proj_ps else "ctx"
                    cC = pool.tile([128, 512], F32, tag=ctag, name=f"cC_{P}")
                    cD = pool.tile([128, 512], F32, tag=ctag, name=f"cD_{P}")
                    cbanks = (cC, cD)
                    osb = rowp.tile([128, 1024], F32, tag="osb", name=f"o_{P}")
                    if ti == n_tiles - 1:
                        # tail: per-row ctx, token-major osb, one contiguous
                        # DMA (the strided form would land unoverlapped here)
                        for rp in range(2):
                            st = 64 * rp
                            for h in range(H):
                                bank = cbanks[h // 8]
                                c0 = 64 * (h % 8)
                                nc.tensor.matmul(
                                    bank[st:st + 64, c0:c0 + 64],
                                    pt[st:st + 64, 64 * h:64 * h + 64],
                                    vb[st:st + 64, h, rp, :],
                                    start=True, stop=True)
                        nc.scalar.copy(osb[:, 0:512], cC[:])
                        nc.vector.tensor_copy(osb[:, 512:1024], cD[:])
                        nc.sync.dma_start(out_d[128 * P:128 * P + 128, :],
                                          osb[:])
                        return
                    # merged-row ctx: one matmul per head covers both rows.
                    # bank[64*(h%2)+q, 128*((h%8)//2) + 64*two + hd]
                    for h in range(H):
                        bank = cbanks[h // 8]
                        st = 64 * (h % 2)
                        c0 = 128 * ((h % 8) // 2)
                        nc.tensor.matmul(
                            bank[st:st + 64, c0:c0 + 128],
                            pt[:, 64 * h:64 * h + 64],
                            vb[:, h, :, :],
                            start=True, stop=True)

                    nc.scalar.copy(osb[:, 0:512], cC[:])
                    nc.vector.tensor_copy(osb[:, 512:1024], cD[:])
                    # osb[64*hpar+q, 512*d + 128*hgrp + 64*two + hd]
                    #   = ctx for head h = 8*d + 2*hgrp + hpar, token 64*two+q
                    ov = out_d[128 * P:128 * P + 128, :].rearrange(
                        "(two q) (d hgrp hpar hd) -> hpar q d hgrp two hd",
                        two=2, q=64, d=2, hgrp=4, hpar=2, hd=64)
                    for hpar in range(2):
                        nc.sync.dma_start(
                            ov[hpar],
                            osb[64 * hpar:64 * hpar + 64, :]
                            .rearrange("q (d hgrp two hd) -> q d hgrp two hd",
                                       d=2, hgrp=4, two=2, hd=64))

                if la is None:   # split mode: caller interleaves
                    return ([lambda pr=pr: att_a(pr) for pr in range(PPT)],
                            [lambda pr=pr: att_b(pr) for pr in range(PPT)])
                units = [lambda pr=pr: att_a(pr) for pr in range(la)]
                for pr in range(la, PPT):
                    units.append(lambda pr=pr: att_a(pr))
                    units.append(lambda pr=pr: att_b(pr - la))
                for pr in range(PPT - la, PPT):
                    units.append(lambda pr=pr: att_b(pr))
                return units

            # ---------------- main schedule ----------------
            # weights (+ tile-0 x interleaved)
            xt0 = [xtp.tile([128, T], BF16, tag=f"xt{d}", name=f"xt{d}_0")
                   for d in range(8)]
            for dch in range(8):
                emit_w("q", wq_d, [dch])
                nc.sync.dma_start(xt0[dch][:],
                                  xt_d[128 * dch:128 * dch + 128, 0:T])
            emit_w("k", wk_d, range(8))
            emit_w("v", wv_d, range(8), eng=nc.gpsimd)

            def interleave(groups, units):
                ui = 0
                for gi, g in enumerate(groups):
                    g()
                    while (ui < len(units)
                           and (gi + 1) * len(units) // len(groups) > ui):
                        units[ui]()
                        ui += 1
                while ui < len(units):
                    units[ui]()
                    ui += 1

            prev_units = []
            for ti in range(n_tiles - 1):
                xt = xt0 if ti == 0 else emit_xt(ti)
                qbd, kt, vts, groups = make_proj(ti, xt)
                interleave(groups, prev_units)
                prev_units = make_att(ti, qbd, kt, vts, la=1)

            # Last tile: fold its own attention into the tail of its
            # projection emission (scores can start as soon as q/k are out;
            # ctx of pair p only needs v subtile p).
            xt = emit_xt(n_tiles - 1)
            qbd, kt, vts, groups = make_proj(n_tiles - 1, xt)
            qk_groups, v_groups = groups[:16], groups[16:]
            interleave(qk_groups, prev_units)
            As, Bs = make_att(n_tiles - 1, qbd, kt, vts, la=None)
            tail = []
            for pr in range(PPT):
                tail.append(As[pr])
                tail.extend(v_groups[2 * pr:2 * pr + 2])
                if pr >= 1:
                    tail.append(Bs[pr - 1])
            tail.append(Bs[PPT - 1])
            for u in tail:
                u()

    return dict(out=out_d)


def _prepare_shards(hidden_states, attention_mask, sim_graph, Wq, bq, Wk, bk,
                    Wv, bv, n_cores=N_CORES):
    import ml_dtypes
    bf = ml_dtypes.bfloat16
    b, m, seq, dim = hidden_states.shape
    R = b * seq
    hs = np.transpose(np.asarray(hidden_states), (0, 2, 1, 3)).reshape(R, m, dim)
    am = np.transpose(np.asarray(attention_mask), (0, 2, 1)).reshape(R, m)
    simT = np.ascontiguousarray(
        np.transpose(np.asarray(sim_graph), (0, 2, 1)), dtype=np.float32)
    WqT = np.ascontiguousarray(np.asarray(Wq).T * 0.125).astype(bf)
    WkT = np.ascontiguousarray(np.asarray(Wk).T).astype(bf)
    WvT = np.ascontiguousarray(np.asarray(Wv).T).astype(bf)
    hones = np.zeros((128, 128), np.float32)
    hones[0:64, 0:64] = 1.0
    hones[64:128, 64:128] = 1.0
    hones = hones.astype(bf)
    rows_per_core = R // n_cores
    # mcol[p, P]: p<64 -> -1e4*(1-am[2P, j=p]); p>=64 -> row 2P+1
    mask = (NEG * (1.0 - am)).astype(np.float32)       # [R, M]
    in_maps = []
    for c in range(n_cores):
        r0 = c * rows_per_core
        xT = np.ascontiguousarray(
            hs[r0:r0 + rows_per_core].reshape(rows_per_core * m, dim).T).astype(bf)
        mk = mask[r0:r0 + rows_per_core]               # [128, 64]
        mcol = np.concatenate([mk[0::2].T, mk[1::2].T], axis=0)  # [128, 64]
        in_maps.append(dict(
            xT=xT,
            simT=simT[r0:r0 + rows_per_core],
            mcol=np.ascontiguousarray(mcol, np.float32),
            WqT=WqT, WkT=WkT, WvT=WvT,
            bq=np.ascontiguousarray(np.asarray(bq) * 0.125, np.float32),
            bk=np.ascontiguousarray(bk, np.float32),
            bv=np.ascontiguousarray(bv, np.float32),
            hones=hones,
        ))
    return in_maps


_CACHE = {}


def _get_compiled(use_bq=False, use_bk=False, use_bv=False):
    key = ("nc3", use_bq, use_bk, use_bv)
    if key not in _CACHE:
        nc = bacc.Bacc("TRN2", target_bir_lowering=False, debug=False)
        build_core_kernel(nc, use_bq=use_bq, use_bk=use_bk, use_bv=use_bv)
        nc.compile()
        _CACHE[key] = nc
    return _CACHE[key]


LAST_EXEC_NS = [None]


def kernel(hidden_states, attention_mask, sim_graph, Wq, bq, Wk, bk, Wv, bv,
           b=4, m=64, seq=256, dim=1024, **_):
    import os
    from concourse.bass_utils import run_bass_kernel_spmd

    nc = _get_compiled(use_bq=bool(np.any(np.asarray(bq))),
                       use_bk=bool(np.any(np.asarray(bk))),
                       use_bv=bool(np.any(np.asarray(bv))))
    in_maps = _prepare_shards(hidden_states, attention_mask, sim_graph,
                              Wq, bq, Wk, bk, Wv, bv)
    trace = bool(int(os.environ.get("BERT_TRACE", "0")))
    if trace:
        try:
            from antenv.axon_hooks import (get_axon_ntff_profile_hook,
                                           set_axon_ntff_profile_hook)
            if get_axon_ntff_profile_hook() is None:
                from trn_agent_boot.trn_boot import _ntff_profile_via_ctypes
                set_axon_ntff_profile_hook(
                    _ntff_profile_via_ctypes("/opt/axon/libaxon_pjrt.so"))
        except Exception:
            trace = False
    res = run_bass_kernel_spmd(nc, in_maps, list(range(N_CORES)), trace=trace)
    LAST_EXEC_NS[0] = res.exec_time_ns
    R = int(b) * int(seq)
    out = np.concatenate([res.results[c]["out"] for c in range(N_CORES)], axis=0)
    return out.reshape(R, int(m), int(dim))


# revision 3
# speedup vs baseline: 1.0586x; 1.0067x over previous
"""Trainium2 Bass kernel for BertSimSelfAttention — v2 (pair-row layout).

Problem (full): B=4, M=64, SEQ=256, DIM=1024, H=16, HD=64.
R = B*SEQ = 1024 rows, data-parallel 128 rows/core x 8 cores.

v2 design vs v1:
  - x and W shipped bf16 (halved DMA, no on-device f32r converts, FWL
    eligible stationaries for the projections).
  - Attention processes ROW PAIRS with row-parity partition strips:
    S'/e/pt for row r live at partitions 64*(r%2)..+64, all 16 heads on
    the free axis (2 PSUM banks per pair). v is consumed at its natural
    partition strip — the per-row v duplication DMA (33.5 MB) is gone.
  - Scores: one matmul per (row, head-pair): stationary = kt[hp] full
    [128,64] (both heads' dims), moving = zero-padded qbd [128,(2,64)]
    so each head contracts only its own 64 dims. Halves scores LDWEIGHTS
    columns and instruction count.
  - Output: heads land in natural order; one contiguous [128, 4KB] DMA
    per pair.
  - Softmax: t = S'*simT (DVE), e = exp(t + mcol) (ACT, mask bias per
    partition, host-built mcol), denominators via halfones matmul,
    reciprocal_approx_fast in PSUM, pt = e*recip (DVE, bf16).
"""

import sys

sys.path.insert(0, "/opt/trn_rl_repo")

import numpy as np
import concourse.bass as bass
import concourse.bacc as bacc
import concourse.mybir as mybir
import concourse.tile as tile

F32 = mybir.dt.float32
BF16 = mybir.dt.bfloat16
AF = mybir.ActivationFunctionType
ALU = mybir.AluOpType

N_CORES = 8
M = 64
DIM = 1024
H = 16
HD = 64
NEG = -10000.0


def build_core_kernel(nc, n_tiles=16, rows_per_tile=8, debug=False,
                      use_bq=False, use_bk=False, use_bv=False):
    assert rows_per_tile % 2 == 0
    T = rows_per_tile * M              # tokens per tile (512)
    n_rows = n_tiles * rows_per_tile
    n_tok = n_rows * M
    n_pairs = n_rows // 2
    PPT = rows_per_tile // 2           # pairs per tile
    SUB = T // 128                     # 128-token subtiles per tile

    xt_d = nc.dram_tensor("xT", (DIM, n_tok), BF16, kind="ExternalInput")
    sim_d = nc.dram_tensor("simT", (n_rows, M, M), F32, kind="ExternalInput")
    mcol_d = nc.dram_tensor("mcol", (128, n_pairs), F32, kind="ExternalInput")
    wq_d = nc.dram_tensor("WqT", (DIM, DIM), BF16, kind="ExternalInput")
    wk_d = nc.dram_tensor("WkT", (DIM, DIM), BF16, kind="ExternalInput")
    wv_d = nc.dram_tensor("WvT", (DIM, DIM), BF16, kind="ExternalInput")
    bq_d = nc.dram_tensor("bq", (DIM,), F32, kind="ExternalInput")
    bk_d = nc.dram_tensor("bk", (DIM,), F32, kind="ExternalInput")
    bv_d = nc.dram_tensor("bv", (DIM,), F32, kind="ExternalInput")
    hones_d = nc.dram_tensor("hones", (128, 128), BF16, kind="ExternalInput")
    out_d = nc.dram_tensor("out", (n_tok, DIM), F32, kind="ExternalOutput")

    dbg = {}
    if debug:
        dbg["qbd"] = nc.dram_tensor("dbg_qbd", (8, 128, 2 * T), F32,
                                    kind="ExternalOutput")
        dbg["kt"] = nc.dram_tensor("dbg_kt", (8, 128, T), F32,
                                   kind="ExternalOutput")
        dbg["v"] = nc.dram_tensor("dbg_v", (SUB, 128, DIM), F32,
                                  kind="ExternalOutput")
        dbg["et"] = nc.dram_tensor("dbg_et", (n_pairs, 128, 1024), F32,
                                   kind="ExternalOutput")
        dbg["pt"] = nc.dram_tensor("dbg_pt", (n_pairs, 128, 1024), F32,
                                   kind="ExternalOutput")

    with tile.TileContext(nc) as tc:
        with (
            tc.tile_pool(name="consts", bufs=1) as consts,
            tc.tile_pool(name="xtp", bufs=2) as xtp,
            tc.tile_pool(name="qbdp", bufs=1) as qbdp,
            tc.tile_pool(name="ktp", bufs=2) as ktp,
            tc.tile_pool(name="vp", bufs=2) as vp,
            tc.tile_pool(name="rowp", bufs=2) as rowp,
            tc.tile_pool(name="proj_ps", bufs=3, space="PSUM") as proj_ps,
            tc.tile_pool(name="att_ps", bufs=3, space="PSUM") as att_ps,
            tc.tile_pool(name="ctx_ps", bufs=2, space="PSUM") as ctx_ps,
        ):
            # ---------------- consts ----------------
            hones = consts.tile([128, 128], BF16)
            nc.sync.dma_start(hones[:], hones_d[:])

            mcol = consts.tile([128, n_pairs], F32)
            nc.sync.dma_start(mcol[:], mcol_d[:])

            if use_bq or use_bk:
                bq_sb = consts.tile([128, 8], F32)
                bk_sb = consts.tile([128, 8], F32)
                nc.sync.dma_start(bq_sb[:], bq_d[:].rearrange("(o p) -> p o", p=128))
                nc.sync.dma_start(bk_sb[:], bk_d[:].rearrange("(o p) -> p o", p=128))
            if use_bv:
                ones_f = consts.tile([1, 128], F32)
                nc.gpsimd.memset(ones_f[:], 1.0)
                ones_b = consts.tile([1, 128], BF16)
                nc.vector.tensor_copy(ones_b[:], ones_f[:])
                bv_row = consts.tile([1, DIM], F32)
                nc.sync.dma_start(bv_row[:], bv_d[:].rearrange("(a o) -> a o", a=1))
                bv_b = consts.tile([1, DIM], BF16)
                nc.vector.tensor_copy(bv_b[:], bv_row[:])

            # qbd: persistent ping/pong zero-padded tiles. The off-diagonal
            # quadrants are zeroed once here; projections only ever write the
            # two valid quadrants, so the zeros persist across tiles.
            qbd_sets = [[qbdp.tile([128, 2 * T], BF16, tag=f"q{hp}{s}",
                                   name=f"qbd{hp}_{s}") for hp in range(8)]
                        for s in range(2)]
            for s in range(2):
                for hp in range(8):
                    nc.gpsimd.memset(qbd_sets[s][hp][:], 0.0)

            # v_bd: per-subtile zero-padded v for merged-row ctx matmuls.
            # Layout [128 (tok: row0 strip0 / row1 strip1), 16 h, 2 two, 64]:
            # quadrant (strip0, two=1) and (strip1, two=0) stay zero.
            vbd_sets = [[qbdp.tile([128, H * 128], BF16, tag=f"vb{sub}{s}",
                                   name=f"vbd{sub}_{s}") for sub in range(SUB)]
                        for s in range(2)]
            for s in range(2):
                for sub in range(SUB):
                    nc.gpsimd.memset(vbd_sets[s][sub][:], 0.0)

            # ---------------- emit helpers ----------------
            def emit_xt(ti):
                t0 = ti * T
                xt = [xtp.tile([128, T], BF16, tag=f"xt{d}", name=f"xt{d}_{ti}")
                      for d in range(8)]
                for dch in range(8):
                    nc.sync.dma_start(
                        xt[dch][:], xt_d[128 * dch:128 * dch + 128, t0:t0 + T])
                return xt

            wts = {name: [consts.tile([128, DIM], BF16, tag=f"w{name}{d}",
                                      name=f"w{name}{d}") for d in range(8)]
                   for name in ("q", "k", "v")}

            def emit_w(name, w_d, dchs, eng=None):
                wt = wts[name]
                for dch in dchs:
                    (eng or nc.sync).dma_start(
                        wt[dch][:], w_d[128 * dch:128 * dch + 128, :])

            def make_proj(ti, xt):
                qbd = qbd_sets[ti % 2]
                kt = [ktp.tile([128, T], BF16, tag=f"k{hp}",
                               name=f"kt{hp}_{ti}") for hp in range(8)]
                vts = vbd_sets[ti % 2]
                groups = []

                def q_group(och):
                    ps = proj_ps.tile([128, T], F32, tag="proj",
                                      name=f"qps{och}_{ti}")
                    for dch in range(8):
                        nc.tensor.matmul(
                            ps[:], wts["q"][dch][:, 128 * och:128 * och + 128],
                            xt[dch][:], start=(dch == 0), stop=(dch == 7))
                    dst = qbd[och]
                    if use_bq:
                        nc.scalar.activation(dst[0:64, 0:T], ps[0:64, :],
                                             AF.Identity,
                                             bias=bq_sb[0:64, och:och + 1],
                                             scale=1.0)
                        nc.vector.tensor_tensor(
                            dst[64:128, T:2 * T], ps[64:128, :],
                            bq_sb[64:128, och:och + 1]
                            .rearrange("p a -> p a").broadcast_to([64, T]),
                            op=ALU.add)
                    else:
                        nc.scalar.copy(dst[0:64, 0:T], ps[0:64, :])
                        nc.vector.tensor_copy(dst[64:128, T:2 * T], ps[64:128, :])

                def k_group(och):
                    ps = proj_ps.tile([128, T], F32, tag="proj",
                                      name=f"kps{och}_{ti}")
                    for dch in range(8):
                        nc.tensor.matmul(
                            ps[:], wts["k"][dch][:, 128 * och:128 * och + 128],
                            xt[dch][:], start=(dch == 0), stop=(dch == 7))
                    if use_bk:
                        nc.scalar.activation(kt[och][:], ps[:], AF.Identity,
                                             bias=bk_sb[:, och:och + 1],
                                             scale=1.0)
                    else:
                        nc.scalar.copy(kt[och][:], ps[:])

                def v_group(sub, oh):
                    ps = proj_ps.tile([128, 512], F32, tag="proj",
                                      name=f"vps{sub}{oh}_{ti}")
                    sl = slice(512 * oh, 512 * oh + 512)
                    for dch in range(8):
                        nc.tensor.matmul(
                            ps[:], xt[dch][:, 128 * sub:128 * sub + 128],
                            wts["v"][dch][:, sl],
                            start=(dch == 0), stop=(dch == 7) and not use_bv)
                    if use_bv:
                        nc.tensor.matmul(ps[:], ones_b[:], bv_b[:, sl],
                                         start=False, stop=True)
                    vb = vts[sub][:].rearrange("p (h two hd) -> p h two hd",
                                               two=2, hd=64)
                    hsl = slice(8 * oh, 8 * oh + 8)
                    nc.scalar.copy(
                        vb[0:64, hsl, 0, :],
                        ps[0:64, :].rearrange("p (h hd) -> p h hd", hd=64))
                    nc.vector.tensor_copy(
                        vb[64:128, hsl, 1, :],
                        ps[64:128, :].rearrange("p (h hd) -> p h hd", hd=64))

                for och in range(8):
                    groups.append(lambda och=och: q_group(och))
                for och in range(8):
                    groups.append(lambda och=och: k_group(och))
                for sub in range(SUB):
                    for oh in range(2):
                        groups.append(lambda sub=sub, oh=oh: v_group(sub, oh))
                return qbd, kt, vts, groups

            # ---------------- attention (pair units) ----------------
            def make_att(ti, qbd, kt, vts, la=1):
                state = {}

                def att_a(pr):
                    P = ti * PPT + pr          # global pair index

                    simP = rowp.tile([128, M], F32, tag="sim", name=f"sim_{P}")
                    nc.sync.dma_start(simP[0:64, :], sim_d[2 * P, :, :])
                    nc.sync.dma_start(simP[64:128, :], sim_d[2 * P + 1, :, :])

                    sA = att_ps.tile([128, 512], F32, tag="att", name=f"sA_{P}")
                    sB = att_ps.tile([128, 512], F32, tag="att", name=f"sB_{P}")
                    banks = (sA, sB)
                    for rp in range(2):
                        rr = 2 * pr + rp
                        tsl = slice(M * rr, M * rr + M)
                        q2 = slice(2 * M * rr, 2 * M * rr + M)  # unused; doc
                        for hp in range(8):
                            bank = banks[hp // 4]
                            c0 = 128 * (hp % 4)
                            nc.tensor.matmul(
                                bank[64 * rp:64 * rp + 64, c0:c0 + 128],
                                kt[hp][:, tsl],
                                qbd[hp][:]
                                .rearrange("p (two t) -> p two t", two=2)
                                [:, :, tsl],
                                start=True, stop=True)

                    # t = S' * simT ; e = exp(t + mcol)
                    tt = rowp.tile([128, 1024], F32, tag="tt", name=f"tt_{P}")
                    et = rowp.tile([128, 1024], BF16, tag="et", name=f"et_{P}")
                    for b in range(2):
                        nc.vector.tensor_tensor(
                            tt[:, 512 * b:512 * b + 512]
                            .rearrange("p (a j) -> p a j", j=M),
                            banks[b][:].rearrange("p (a j) -> p a j", j=M),
                            simP[:].rearrange("p (a j) -> p a j", a=1)
                            .broadcast_to([128, 8, M]),
                            op=ALU.mult)
                    nc.scalar.activation(et[:], tt[:], AF.Exp,
                                         bias=mcol[:, P:P + 1])

                    # NOTE: simP is indexed [j, q] per strip; matches S'[j, q].
                    dnA = att_ps.tile([128, 512], F32, tag="att", name=f"dnA_{P}")
                    dnB = att_ps.tile([128, 512], F32, tag="att", name=f"dnB_{P}")
                    for b, dn in enumerate((dnA, dnB)):
                        nc.tensor.matmul(dn[:], hones[:],
                                         et[:, 512 * b:512 * b + 512],
                                         start=True, stop=True)
                        nc.vector.reciprocal_approx_fast(out=dn[:], in_=dn[:])

                    pt = rowp.tile([128, 1024], BF16, tag="pt", name=f"pt_{P}")
                    for b, dn in enumerate((dnA, dnB)):
                        nc.vector.tensor_tensor(pt[:, 512 * b:512 * b + 512],
                                                et[:, 512 * b:512 * b + 512],
                                                dn[:], op=ALU.mult)
                    if debug:
                        dsb = rowp.tile([128, 1024], F32, tag="dbg",
                                        name=f"de_{P}")
                        nc.scalar.copy(dsb[:], et[:])
                        nc.gpsimd.dma_start(dbg["et"][P, :, :], dsb[:])
                        dpb = rowp.tile([128, 1024], F32, tag="dbg2",
                                        name=f"dp_{P}")
                        nc.scalar.copy(dpb[:], pt[:])
                        nc.gpsimd.dma_start(dbg["pt"][P, :, :], dpb[:])
                    state[pr] = pt

                def att_b(pr):
                    P = ti * PPT + pr
                    pt = state.pop(pr)
                    vb = vts[pr][:].rearrange("p (h two hd) -> p h two hd",
                                              two=2, hd=64)
                    # in the last tile the proj psum pool is idle — borrow it
                    # so consecutive pairs' ctx don't serialize on 2 banks
                    pool = (proj_ps if (ti == n_tiles - 1 and pr % 2 == 1)
                            else ctx_ps)
                    ctag = "proj" if pool is proj_ps else "ctx"
                    cC = pool.tile([128, 512], F32, tag=ctag, name=f"cC_{P}")
                    cD = pool.tile([128, 512], F32, tag=ctag, name=f"cD_{P}")
                    cbanks = (cC, cD)
                    osb = rowp.tile([128, 1024], F32, tag="osb", name=f"o_{P}")
                    if ti == n_tiles - 1:
                        # tail: per-row ctx, token-major osb, one contiguous
                        # DMA (the strided form would land unoverlapped here)
                        for rp in range(2):
                            st = 64 * rp
                            for h in range(H):
                                bank = cbanks[h // 8]
                                c0 = 64 * (h % 8)
                                nc.tensor.matmul(
                                    bank[st:st + 64, c0:c0 + 64],
                                    pt[st:st + 64, 64 * h:64 * h + 64],
                                    vb[st:st + 64, h, rp, :],
                                    start=True, stop=True)
                        nc.scalar.copy(osb[:, 0:512], cC[:])
                        nc.sync.dma_start(
                            out_d[128 * P:128 * P + 128, :]
                            .rearrange("t (a c) -> t a c", a=2)[:, 0, :],
                            osb[:, 0:512])
                        nc.vector.tensor_copy(osb[:, 512:1024], cD[:])
                        nc.sync.dma_start(
                            out_d[128 * P:128 * P + 128, :]
                            .rearrange("t (a c) -> t a c", a=2)[:, 1, :],
                            osb[:, 512:1024])
                        return
                    # merged-row ctx: one matmul per head covers both rows.
                    # bank[64*(h%2)+q, 128*((h%8)//2) + 64*two + hd]
                    for h in range(H):
                        bank = cbanks[h // 8]
                        st = 64 * (h % 2)
                        c0 = 128 * ((h % 8) // 2)
                        nc.tensor.matmul(
                            bank[st:st + 64, c0:c0 + 128],
                            pt[:, 64 * h:64 * h + 64],
                            vb[:, h, :, :],
                            start=True, stop=True)

                    nc.scalar.copy(osb[:, 0:512], cC[:])
                    nc.vector.tensor_copy(osb[:, 512:1024], cD[:])
                    # osb[64*hpar+q, 512*d + 128*hgrp + 64*two + hd]
                    #   = ctx for head h = 8*d + 2*hgrp + hpar, token 64*two+q
                    ov = out_d[128 * P:128 * P + 128, :].rearrange(
                        "(two q) (d hgrp hpar hd) -> hpar q d hgrp two hd",
                        two=2, q=64, d=2, hgrp=4, hpar=2, hd=64)
                    for hpar in range(2):
                        nc.sync.dma_start(
                            ov[hpar],
                            osb[64 * hpar:64 * hpar + 64, :]
                            .rearrange("q (d hgrp two hd) -> q d hgrp two hd",
                                       d=2, hgrp=4, two=2, hd=64))

                if la is None:   # split mode: caller interleaves
                    return ([lambda pr=pr: att_a(pr) for pr in range(PPT)],
                            [lambda pr=pr: att_b(pr) for pr in range(PPT)])
                units = [lambda pr=pr: att_a(pr) for pr in range(la)]
                for pr in range(la, PPT):
                    units.append(lambda pr=pr: att_a(pr))
                    units.append(lambda pr=pr: att_b(pr - la))
                for pr in range(PPT - la, PPT):
                    units.append(lambda pr=pr: att_b(pr))
                return units

            # ---------------- main schedule ----------------
            # weights (+ tile-0 x interleaved)
            xt0 = [xtp.tile([128, T], BF16, tag=f"xt{d}", name=f"xt{d}_0")
                   for d in range(8)]
            for dch in range(8):
                emit_w("q", wq_d, [dch],
                       eng=(nc.sync if dch % 2 == 0 else nc.scalar))
                (nc.scalar if dch % 2 == 0 else nc.sync).dma_start(
                    xt0[dch][:], xt_d[128 * dch:128 * dch + 128, 0:T])
            emit_w("k", wk_d, range(8))
            emit_w("v", wv_d, range(8), eng=nc.gpsimd)

            def interleave(groups, units):
                ui = 0
                for gi, g in enumerate(groups):
                    g()
                    while (ui < len(units)
                           and (gi + 1) * len(units) // len(groups) > ui):
                        units[ui]()
                        ui += 1
                while ui < len(units):
                    units[ui]()
                    ui += 1

            prev_units = []
            for ti in range(n_tiles - 1):
                xt = xt0 if ti == 0 else emit_xt(ti)
                qbd, kt, vts, groups = make_proj(ti, xt)
                interleave(groups, prev_units)
                prev_units = make_att(ti, qbd, kt, vts, la=1)

            # Last tile: fold its own attention into the tail of its
            # projection emission (scores can start as soon as q/k are out;
            # ctx of pair p only needs v subtile p).
            xt = emit_xt(n_tiles - 1)
            qbd, kt, vts, groups = make_proj(n_tiles - 1, xt)
            qk_groups, v_groups = groups[:16], groups[16:]
            interleave(qk_groups, prev_units)
            As, Bs = make_att(n_tiles - 1, qbd, kt, vts, la=None)
            tail = [As[0], v_groups[0], As[1], v_groups[1],
                    As[2], v_groups[2], As[3], v_groups[3],
                    v_groups[4], Bs[0], v_groups[5], v_groups[6],
                    Bs[1], v_groups[7], Bs[2], Bs[3]]
            for u in tail:
                u()

    return dict(out=out_d)


def _prepare_shards(hidden_states, attention_mask, sim_graph, Wq, bq, Wk, bk,
                    Wv, bv, n_cores=N_CORES):
    import ml_dtypes
    bf = ml_dtypes.bfloat16
    b, m, seq, dim = hidden_states.shape
    R = b * seq
    hs = np.transpose(np.asarray(hidden_states), (0, 2, 1, 3)).reshape(R, m, dim)
    am = np.transpose(np.asarray(attention_mask), (0, 2, 1)).reshape(R, m)
    simT = np.ascontiguousarray(
        np.transpose(np.asarray(sim_graph), (0, 2, 1)), dtype=np.float32)
    WqT = np.ascontiguousarray(np.asarray(Wq).T * 0.125).astype(bf)
    WkT = np.ascontiguousarray(np.asarray(Wk).T).astype(bf)
    WvT = np.ascontiguousarray(np.asarray(Wv).T).astype(bf)
    hones = np.zeros((128, 128), np.float32)
    hones[0:64, 0:64] = 1.0
    hones[64:128, 64:128] = 1.0
    hones = hones.astype(bf)
    rows_per_core = R // n_cores
    # mcol[p, P]: p<64 -> -1e4*(1-am[2P, j=p]); p>=64 -> row 2P+1
    mask = (NEG * (1.0 - am)).astype(np.float32)       # [R, M]
    in_maps = []
    for c in range(n_cores):
        r0 = c * rows_per_core
        xT = np.ascontiguousarray(
            hs[r0:r0 + rows_per_core].reshape(rows_per_core * m, dim).T).astype(bf)
        mk = mask[r0:r0 + rows_per_core]               # [128, 64]
        mcol = np.concatenate([mk[0::2].T, mk[1::2].T], axis=0)  # [128, 64]
        in_maps.append(dict(
            xT=xT,
            simT=simT[r0:r0 + rows_per_core],
            mcol=np.ascontiguousarray(mcol, np.float32),
            WqT=WqT, WkT=WkT, WvT=WvT,
            bq=np.ascontiguousarray(np.asarray(bq) * 0.125, np.float32),
            bk=np.ascontiguousarray(bk, np.float32),
            bv=np.ascontiguousarray(bv, np.float32),
            hones=hones,
        ))
    return in_maps


_CACHE = {}


def _get_compiled(use_bq=False, use_bk=False, use_bv=False):
    key = ("nc3", use_bq, use_bk, use_bv)
    if key not in _CACHE:
        nc = bacc.Bacc("TRN2", target_bir_lowering=False, debug=False)
        build_core_kernel(nc, use_bq=use_bq, use_bk=use_bk, use_bv=use_bv)
        nc.compile()
        _CACHE[key] = nc
    return _CACHE[key]


LAST_EXEC_NS = [None]


def kernel(hidden_states, attention_mask, sim_graph, Wq, bq, Wk, bk, Wv, bv,
           b=4, m=64, seq=256, dim=1024, **_):
    import os
    from concourse.bass_utils import run_bass_kernel_spmd

    nc = _get_compiled(use_bq=bool(np.any(np.asarray(bq))),
                       use_bk=bool(np.any(np.asarray(bk))),
                       use_bv=bool(np.any(np.asarray(bv))))
    in_maps = _prepare_shards(hidden_states, attention_mask, sim_graph,
                              Wq, bq, Wk, bk, Wv, bv)
    trace = bool(int(os.environ.get("BERT_TRACE", "0")))
    if trace:
        try:
            from antenv.axon_hooks import (get_axon_ntff_profile_hook,
                                           set_axon_ntff_profile_hook)
            if get_axon_ntff_profile_hook() is None:
                from trn_agent_boot.trn_boot import _ntff_profile_via_ctypes
                set_axon_ntff_profile_hook(
                    _ntff_profile_via_ctypes("/opt/axon/libaxon_pjrt.so"))
        except Exception:
            trace = False
    res = run_bass_kernel_spmd(nc, in_maps, list(range(N_CORES)), trace=trace)
    LAST_EXEC_NS[0] = res.exec_time_ns
    R = int(b) * int(seq)
    out = np.concatenate([res.results[c]["out"] for c in range(N_CORES)], axis=0)
    return out.reshape(R, int(m), int(dim))
